# revision 7
# baseline (speedup 1.0000x reference)
"""Multi-head attention (B=4, S=2048, H=1024, 16 heads) on 8 Trainium2 cores.

Sharding: core c = 2*b + g handles batch b with head-group g (8 heads = 512 of
1024 H-columns).  Each core computes Q/K/V projections for its column slice,
attention for its 8 heads, and a partial output projection against its 512
rows of wo.  The host sums the two partials per batch and adds bo.

v2 structure (per core):
  - x arrives pre-transposed from the host (xT [H, S] bf16): no PE transpose.
  - Scores for a HEAD PAIR run as two concurrent row-tiled matmuls
    (tile_position (0,0) / (64,0), K=HD=64 each) into one [128, 2, 512] psum
    pair-tile -> 2x score throughput vs. serial per-head matmuls.
  - One exp ACTIVATE per (pair, kc) covers both heads ([128, 1024]), scale
    folded; the scalar engine is the pacing engine in steady state.
  - v is stored ones-augmented ([.., 65]) so the AV matmul also produces the
    softmax denominator in psum partition 64 (one [K=128, M=65] matmul per
    head per kc; full-array, nothing to tile).
  - Normalization per window: reciprocal_approx_fast on the two denominator
    rows, bf16 round-trip through DRAM for the partition-broadcast, one
    in-place multiply over the pair's 128 ctxT partitions.
  - Projection work (qk of the next pair, v during the first window, o during
    the last pair) is dripped through the attention iterations as PE fill.
PSUM: 2x sc pair-tiles (4 banks) + 2 ctx accumulators + 2 fill accs = 8.
All matmuls bf16 with fp32 psum accumulation.
"""
import sys

if "/opt/trn_rl_repo" not in sys.path:
    sys.path.insert(0, "/opt/trn_rl_repo")

from collections import deque

import numpy as np

import concourse.bass as bass
import concourse.tile as tile
from concourse import bacc, mybir
from concourse.bass_utils import run_bass_kernel_spmd

B, S, H = 4, 2048, 1024
NH, HD = 16, 64
G = H // 2            # local H columns per core
NHL = NH // 2         # local heads per core
P = 128
F32 = mybir.dt.float32
BF16 = mybir.dt.bfloat16
SCALE = 1.0 / float(np.sqrt(HD))

TT = S // P           # 16 token tiles
HC = H // P           # 8 contraction chunks for projections
CT = G // P           # 4 c-tiles == head pairs
KC = S // P           # 16 k chunks
W = 512               # q window width
NQW = S // W          # 4 q windows
NWH = CT * NQW * 2    # 32 (window, head) rows for norm scratch

_NC_CACHE = {}


def _emit(nc, tc, aps, with_bias):
    xT, wq, wk, wv, wo, bq, bk, bv, out, rcp_dram = aps

    import contextlib
    ctx = contextlib.ExitStack()
    with ctx:
        persist = ctx.enter_context(tc.tile_pool(name="persist", bufs=1))

        # ---- persistent sbuf tensors ----
        xT_sb = persist.tile([P, HC, S], BF16)
        qT = persist.tile([P, CT, S], BF16)
        kT = persist.tile([P, CT, S], BF16)
        v_aug = persist.tile([P, KC, NHL, HD + 1], BF16)
        ctxT = persist.tile([P, CT, S], BF16)
        wq_sb = persist.tile([P, HC, G], BF16)
        wk_sb = persist.tile([P, HC, G], BF16)
        wv_sb = persist.tile([P, HC, G], BF16)
        wo_sb = persist.tile([P, CT, H], BF16)
        bq_sb = persist.tile([P, CT], F32)
        bk_sb = persist.tile([P, CT], F32)
        bv_row = persist.tile([1, G], BF16)
        ones_col = persist.tile([1, P], BF16)

        if with_bias:
            nc.vector.memset(ones_col, 1.0)
        nc.vector.memset(v_aug[:, :, :, HD:HD + 1], 1.0)

        # ---- psum pools: 2x[128,2,512] sc + 2x[65,512] ctx + 2x[128,512] ----
        sc_pool = ctx.enter_context(tc.tile_pool(name="sc_ps", bufs=2, space="PSUM"))
        ctx_pool = ctx.enter_context(tc.tile_pool(name="ctx_ps", bufs=1, space="PSUM"))
        fill_pool = ctx.enter_context(tc.tile_pool(name="fill_ps", bufs=2, space="PSUM"))

        # ---- input DMA ----
        if with_bias:
            nc.sync.dma_start(out=bq_sb, in_=bq.rearrange("(ct p) -> p ct", p=P))
            nc.sync.dma_start(out=bk_sb, in_=bk.rearrange("(ct p) -> p ct", p=P))
            bv_f = persist.tile([1, G], F32)
            nc.sync.dma_start(out=bv_f, in_=bv.rearrange("(a c) -> a c", a=1))
            nc.vector.tensor_copy(out=bv_row, in_=bv_f)

        nc.sync.dma_start(out=wk_sb, in_=wk.rearrange("(hc p) c -> p hc c", p=P))
        # xT staged in four 512-token column slices so lead-in matmuls start early
        for nq in range(4):
            nc.sync.dma_start(
                out=xT_sb[:, :, nq * 512:(nq + 1) * 512],
                in_=xT.rearrange("(hc p) s -> p hc s", p=P)[
                    :, :, nq * 512:(nq + 1) * 512])
        nc.sync.dma_start(out=wq_sb, in_=wq.rearrange("(hc p) c -> p hc c", p=P))
        nc.sync.dma_start(out=wv_sb, in_=wv.rearrange("(hc p) c -> p hc c", p=P))
        nc.sync.dma_start(out=wo_sb, in_=wo.rearrange("(cc p) o -> p cc o", p=P))

        # ---------- fill machinery: single-matmul closures + evacs ----------
        osb = ctx.enter_context(tc.tile_pool(name="o_sb", bufs=4))
        fill_q = deque()

        def enq_qk_group(which, ct, nq):
            w_sb, b_sb, dst = ((wq_sb, bq_sb, qT), (wk_sb, bk_sb, kT))[which]
            state = {}

            def mk(hc):
                def emit():
                    if hc == 0:
                        state["acc"] = fill_pool.tile(
                            [P, W], F32, tag="acc", name="qkacc")
                    nc.tensor.matmul(
                        state["acc"],
                        lhsT=w_sb[:, hc, ct * P:(ct + 1) * P],
                        rhs=xT_sb[:, hc, nq * W:(nq + 1) * W],
                        start=(hc == 0), stop=(hc == HC - 1))
                    if hc == HC - 1:
                        sl = dst[:, ct, nq * W:(nq + 1) * W]
                        if with_bias:
                            nc.vector.tensor_scalar_add(
                                out=sl, in0=state["acc"],
                                scalar1=b_sb[:, ct:ct + 1])
                        else:
                            nc.vector.tensor_copy(out=sl, in_=state["acc"])
                return emit
            for hc in range(HC):
                fill_q.append(mk(hc))

        def emit_v_now(tt):
            acc = fill_pool.tile([P, G], F32, tag="acc", name="vacc")
            for hc in range(HC):
                nc.tensor.matmul(
                    acc,
                    lhsT=xT_sb[:, hc, tt * P:(tt + 1) * P],
                    rhs=wv_sb[:, hc, :],
                    start=(hc == 0),
                    stop=(not with_bias and hc == HC - 1))
            if with_bias:
                nc.tensor.matmul(
                    acc, lhsT=ones_col, rhs=bv_row, start=False, stop=True)
            nc.vector.tensor_copy(
                out=v_aug[:, tt, :, 0:HD],
                in_=acc.rearrange("p (h d) -> p h d", h=NHL))

        def enq_o_group(tt, no):
            state = {}

            def mk(cc):
                def emit():
                    if cc == 0:
                        state["acc"] = fill_pool.tile(
                            [P, W], F32, tag="acc", name="oacc")
                    nc.tensor.matmul(
                        state["acc"],
                        lhsT=ctxT[:, cc, tt * P:(tt + 1) * P],
                        rhs=wo_sb[:, cc, no * W:(no + 1) * W],
                        start=(cc == 0), stop=(cc == CT - 1))
                    if cc == CT - 1:
                        ot = osb.tile([P, W], BF16)
                        nc.vector.tensor_copy(out=ot, in_=state["acc"])
                        nc.sync.dma_start(
                            out=out[tt * P:(tt + 1) * P,
                                    no * W:(no + 1) * W],
                            in_=ot)
                return emit
            for cc in range(CT):
                fill_q.append(mk(cc))

        def drain_fill(n):
            for _ in range(n):
                if not fill_q:
                    return
                fill_q.popleft()()

        # ---- lead-in: kT + first q window of pair 0 ----
        for nq in range(NQW):
            enq_qk_group(1, 0, nq)
        enq_qk_group(0, 0, 0)
        drain_fill(len(fill_q))

        exp_pool = ctx.enter_context(tc.tile_pool(name="expp", bufs=4))
        norm_pool = ctx.enter_context(tc.tile_pool(name="normp", bufs=2))

        # ---- main attention loop: pair -> q-window -> kc ----
        for pair in range(CT):
            h0 = 2 * pair
            for qw in range(NQW):
                q0 = qw * W
                w_idx = (pair * NQW + qw) * 2
                ctx0 = ctx_pool.tile([HD + 1, W], F32, tag="c0", name="ctx0")
                ctx1 = ctx_pool.tile([HD + 1, W], F32, tag="c1", name="ctx1")

                # enqueue this window's drip work (deadline: all of it
                # drains before the window ends).  Own next-qT; the next
                # pair's kT spread over the middle windows; its first qT in
                # the last window; o-projection during the last pair.
                if qw + 1 < NQW:
                    enq_qk_group(0, pair, qw + 1)
                if pair + 1 < CT:
                    kt_sched = {1: (0, 1), 2: (2, 3)} if pair == 0 else \
                        {0: (0,), 1: (1,), 2: (2, 3)}
                    for nq in kt_sched.get(qw, ()):
                        enq_qk_group(1, pair + 1, nq)
                    if qw == NQW - 1:
                        enq_qk_group(0, pair + 1, 0)
                if pair == CT - 1 and qw > 0:
                    for tt in range(4 * (qw - 1), 4 * qw):
                        for no in range(H // W):
                            enq_o_group(tt, no)

                budget = max(1, -(-len(fill_q) // KC))

                first_win = (pair == 0 and qw == 0)
                prev_ex = None
                for kc in range(KC):
                    if first_win:
                        # v[kc] must exist before AV(kc) next iteration.
                        # fill groups must not interleave with v groups on
                        # the shared psum "acc" slots, so the one queued qT
                        # group drains as a single lump between two of them.
                        emit_v_now(kc)
                        if kc == 5:
                            drain_fill(HC)
                    sc = sc_pool.tile([P, 2, W], F32, tag="sc", name="sc")
                    nc.tensor.matmul(
                        sc[:, 0, :],
                        lhsT=kT[0:HD, pair, kc * P:(kc + 1) * P],
                        rhs=qT[0:HD, pair, q0:q0 + W],
                        start=True, stop=True)
                    nc.tensor.matmul(
                        sc[:, 1, :],
                        lhsT=kT[HD:P, pair, kc * P:(kc + 1) * P],
                        rhs=qT[HD:P, pair, q0:q0 + W],
                        start=True, stop=True)
                    if not first_win:
                        drain_fill(budget)
                    if prev_ex is not None:
                        pex, pkc = prev_ex
                        nc.tensor.matmul(
                            ctx0, lhsT=v_aug[:, pkc, h0, :],
                            rhs=pex[:, 0, :],
                            start=(pkc == 0), stop=False)
                        nc.tensor.matmul(
                            ctx1, lhsT=v_aug[:, pkc, h0 + 1, :],
                            rhs=pex[:, 1, :],
                            start=(pkc == 0), stop=False)
                    ex = exp_pool.tile([P, 2, W], BF16, tag="ex", name="ex")
                    nc.scalar.activation(
                        out=ex, in_=sc,
                        func=mybir.ActivationFunctionType.Exp,
                        scale=SCALE)
                    prev_ex = (ex, kc)

                pex, pkc = prev_ex
                nc.tensor.matmul(
                    ctx0, lhsT=v_aug[:, pkc, h0, :], rhs=pex[:, 0, :],
                    start=False, stop=True)
                nc.tensor.matmul(
                    ctx1, lhsT=v_aug[:, pkc, h0 + 1, :], rhs=pex[:, 1, :],
                    start=False, stop=True)

                # ---- window wrap-up: evac raw ctx, reciprocal of the two
                # denominator rows (straight from psum partition 64; DVE
                # outputs must be 32-partition aligned), DRAM round trip
                # for the partition-broadcast, one in-place multiply ----
                rcp2f = norm_pool.tile([33, W], F32, tag="rcpf")
                nc.vector.reciprocal(
                    out=rcp2f[0:1, :], in_=ctx0[HD:HD + 1, :])
                nc.vector.reciprocal(
                    out=rcp2f[32:33, :], in_=ctx1[HD:HD + 1, :])
                nc.vector.tensor_copy(
                    out=ctxT[0:HD, pair, q0:q0 + W], in_=ctx0[0:HD, :])
                nc.vector.tensor_copy(
                    out=ctxT[HD:P, pair, q0:q0 + W], in_=ctx1[0:HD, :])
                rcp2 = norm_pool.tile([33, W], BF16, tag="rcpb")
                nc.vector.tensor_copy(out=rcp2[0:1, :], in_=rcp2f[0:1, :])
                nc.vector.tensor_copy(
                    out=rcp2[32:33, :], in_=rcp2f[32:33, :])
                nc.sync.dma_start(
                    out=rcp_dram[w_idx:w_idx + 1, :], in_=rcp2[0:1, :])
                nc.sync.dma_start(
                    out=rcp_dram[w_idx + 1:w_idx + 2, :], in_=rcp2[32:33, :])
                bcast = norm_pool.tile([P, W], BF16, tag="bcast")
                for hh in range(2):
                    row = rcp_dram[w_idx + hh:w_idx + hh + 1, :]
                    nc.sync.dma_start(
                        out=bcast[hh * HD:(hh + 1) * HD, :],
                        in_=bass.AP(tensor=row.tensor, offset=row.offset,
                                    ap=[[0, HD], [1, W]]))
                sl = ctxT[:, pair, q0:q0 + W]
                nc.vector.tensor_mul(out=sl, in0=sl, in1=bcast)

        # ---- tail: o-projection for the last q window ----
        for tt in range(4 * (NQW - 1), 4 * NQW):
            for no in range(H // W):
                enq_o_group(tt, no)
        drain_fill(len(fill_q))


def build_program(with_bias=True):
    if with_bias in _NC_CACHE:
        return _NC_CACHE[with_bias]
    nc = bacc.Bacc("TRN2", debug=False, num_devices=8)
    xT = nc.dram_tensor("xT", [H, S], BF16, kind="ExternalInput").ap()
    wq = nc.dram_tensor("wq", [H, G], BF16, kind="ExternalInput").ap()
    wk = nc.dram_tensor("wk", [H, G], BF16, kind="ExternalInput").ap()
    wv = nc.dram_tensor("wv", [H, G], BF16, kind="ExternalInput").ap()
    wo = nc.dram_tensor("wo", [G, H], BF16, kind="ExternalInput").ap()
    bq = nc.dram_tensor("bq", [G], F32, kind="ExternalInput").ap()
    bk = nc.dram_tensor("bk", [G], F32, kind="ExternalInput").ap()
    bv = nc.dram_tensor("bv", [G], F32, kind="ExternalInput").ap()
    out = nc.dram_tensor("out", [S, H], BF16, kind="ExternalOutput").ap()
    rcp_dram = nc.dram_tensor("rcp_scratch", [NWH, W], BF16).ap()
    with tile.TileContext(nc) as tc:
        _emit(nc, tc, (xT, wq, wk, wv, wo, bq, bk, bv, out, rcp_dram),
              with_bias)
    nc.compile()
    _NC_CACHE[with_bias] = nc
    return nc


def make_in_maps(x, wq, bq, wk, bk, wv, bv, wo, bo):
    import ml_dtypes
    bf = ml_dtypes.bfloat16
    x = np.asarray(x, dtype=np.float32)
    wq, wk, wv, wo = (np.asarray(w, np.float32).astype(bf)
                      for w in (wq, wk, wv, wo))
    in_maps = []
    for c in range(8):
        b, g = divmod(c, 2)
        sl = slice(g * G, (g + 1) * G)
        in_maps.append({
            "xT": np.ascontiguousarray(x[b].T.astype(bf)),
            "wq": np.ascontiguousarray(wq[:, sl]),
            "wk": np.ascontiguousarray(wk[:, sl]),
            "wv": np.ascontiguousarray(wv[:, sl]),
            "wo": np.ascontiguousarray(wo[sl, :]),
            "bq": np.ascontiguousarray(np.asarray(bq, np.float32)[sl]),
            "bk": np.ascontiguousarray(np.asarray(bk, np.float32)[sl]),
            "bv": np.ascontiguousarray(np.asarray(bv, np.float32)[sl]),
        })
    return in_maps


def gather_out(results, bo):
    bo = np.asarray(bo, dtype=np.float32)
    out = np.empty((B, S, H), dtype=np.float32)
    for b in range(B):
        out[b] = (results[2 * b]["out"].astype(np.float32)
                  + results[2 * b + 1]["out"].astype(np.float32) + bo)
    return out


def kernel(x, wq, bq, wk, bk, wv, bv, wo, bo, trace=False):
    with_bias = any(
        np.any(np.asarray(b)) for b in (bq, bk, bv))
    nc = build_program(with_bias)
    in_maps = make_in_maps(x, wq, bq, wk, bk, wv, bv, wo, bo)
    r = run_bass_kernel_spmd(nc, in_maps, list(range(8)), trace=trace)
    out = gather_out(r.results, bo)
    if trace:
        kernel.last_exec_time_ns = r.exec_time_ns
        kernel.last_results = r
    return out


# revision 10
# speedup vs baseline: 1.2835x; 1.2835x over previous
"""Multi-head attention (B=4, S=2048, H=1024, 16 heads) on 8 Trainium2 cores.

Sharding: core c = 2*b + g handles batch b with head-group g (8 heads = 512 of
1024 H-columns).  Each core computes Q/K/V projections for its column slice,
attention for its 8 heads, and a partial output projection against its 512
rows of wo.  The host sums the two partials per batch and adds bo.

v2 structure (per core):
  - x arrives pre-transposed from the host (xT [H, S] bf16): no PE transpose.
  - Scores for a HEAD PAIR run as two concurrent row-tiled matmuls
    (tile_position (0,0) / (64,0), K=HD=64 each) into one [128, 2, 512] psum
    pair-tile -> 2x score throughput vs. serial per-head matmuls.
  - One exp ACTIVATE per (pair, kc) covers both heads ([128, 1024]), scale
    folded; the scalar engine is the pacing engine in steady state.
  - v is stored ones-augmented ([.., 65]) so the AV matmul also produces the
    softmax denominator in psum partition 64 (one [K=128, M=65] matmul per
    head per kc; full-array, nothing to tile).
  - Normalization per window: reciprocal_approx_fast on the two denominator
    rows, bf16 round-trip through DRAM for the partition-broadcast, one
    in-place multiply over the pair's 128 ctxT partitions.
  - Projection work (qk of the next pair, v during the first window, o during
    the last pair) is dripped through the attention iterations as PE fill.
PSUM: 2x sc pair-tiles (4 banks) + 2 ctx accumulators + 2 fill accs = 8.
All matmuls bf16 with fp32 psum accumulation.
"""
import sys

if "/opt/trn_rl_repo" not in sys.path:
    sys.path.insert(0, "/opt/trn_rl_repo")

from collections import deque

import numpy as np

import concourse.bass as bass
import concourse.tile as tile
from concourse import bacc, mybir
from concourse.bass_utils import run_bass_kernel_spmd

B, S, H = 4, 2048, 1024
NH, HD = 16, 64
G = H // 2            # local H columns per core
NHL = NH // 2         # local heads per core
P = 128
F32 = mybir.dt.float32
BF16 = mybir.dt.bfloat16
SCALE = 1.0 / float(np.sqrt(HD))

TT = S // P           # 16 token tiles
HC = H // P           # 8 contraction chunks for projections
CT = G // P           # 4 c-tiles == head pairs
KC = S // P           # 16 k chunks
W = 512               # q window width
NQW = S // W          # 4 q windows
NWH = CT * NQW * 2    # 32 (window, head) rows for norm scratch

_NC_CACHE = {}


def _emit(nc, tc, aps, with_bias):
    xT, wq, wk, wv, wo, bq, bk, bv, out, rcp_dram = aps

    import contextlib
    ctx = contextlib.ExitStack()
    with ctx:
        persist = ctx.enter_context(tc.tile_pool(name="persist", bufs=1))

        # ---- persistent sbuf tensors ----
        xT_sb = persist.tile([P, HC, S], BF16)
        qT = persist.tile([P, CT, S], BF16)
        kT = persist.tile([P, CT, S], BF16)
        v_aug = persist.tile([P, KC, NHL, HD + 1], BF16)
        ctxT = persist.tile([P, CT, S], BF16)
        wq_sb = persist.tile([P, HC, G], BF16)
        wk_sb = persist.tile([P, HC, G], BF16)
        wv_sb = persist.tile([P, HC, G], BF16)
        wo_sb = persist.tile([P, CT, H], BF16)
        bq_sb = persist.tile([P, CT], F32)
        bk_sb = persist.tile([P, CT], F32)
        bv_row = persist.tile([1, G], BF16)
        ones_col = persist.tile([1, P], BF16)

        if with_bias:
            nc.vector.memset(ones_col, 1.0)
        nc.vector.memset(v_aug[:, :, :, HD:HD + 1], 1.0)

        # ---- psum pools: 2x[128,2,512] sc + 2x[65,512] ctx + 2x[128,512] ----
        sc_pool = ctx.enter_context(tc.tile_pool(name="sc_ps", bufs=2, space="PSUM"))
        ctx_pool = ctx.enter_context(tc.tile_pool(name="ctx_ps", bufs=1, space="PSUM"))
        fill_pool = ctx.enter_context(tc.tile_pool(name="fill_ps", bufs=2, space="PSUM"))

        # ---- input DMA ----
        if with_bias:
            nc.sync.dma_start(out=bq_sb, in_=bq.rearrange("(ct p) -> p ct", p=P))
            nc.sync.dma_start(out=bk_sb, in_=bk.rearrange("(ct p) -> p ct", p=P))
            bv_f = persist.tile([1, G], F32)
            nc.sync.dma_start(out=bv_f, in_=bv.rearrange("(a c) -> a c", a=1))
            nc.vector.tensor_copy(out=bv_row, in_=bv_f)

        nc.sync.dma_start(out=wk_sb, in_=wk.rearrange("(hc p) c -> p hc c", p=P))
        # xT staged in four 512-token column slices so lead-in matmuls start early
        for nq in range(4):
            nc.sync.dma_start(
                out=xT_sb[:, :, nq * 512:(nq + 1) * 512],
                in_=xT.rearrange("(hc p) s -> p hc s", p=P)[
                    :, :, nq * 512:(nq + 1) * 512])
        nc.sync.dma_start(out=wq_sb, in_=wq.rearrange("(hc p) c -> p hc c", p=P))
        nc.sync.dma_start(out=wv_sb, in_=wv.rearrange("(hc p) c -> p hc c", p=P))
        nc.sync.dma_start(out=wo_sb, in_=wo.rearrange("(cc p) o -> p cc o", p=P))

        # ---------- fill machinery: single-matmul closures + evacs ----------
        osb = ctx.enter_context(tc.tile_pool(name="o_sb", bufs=4))
        fill_q = deque()

        def enq_qk_group(which, ct, nq):
            w_sb, b_sb, dst = ((wq_sb, bq_sb, qT), (wk_sb, bk_sb, kT))[which]
            state = {}

            def mk(hc):
                def emit():
                    if hc == 0:
                        state["acc"] = fill_pool.tile(
                            [P, W], F32, tag="acc", name="qkacc")
                    nc.tensor.matmul(
                        state["acc"],
                        lhsT=w_sb[:, hc, ct * P:(ct + 1) * P],
                        rhs=xT_sb[:, hc, nq * W:(nq + 1) * W],
                        start=(hc == 0), stop=(hc == HC - 1))
                    if hc == HC - 1:
                        sl = dst[:, ct, nq * W:(nq + 1) * W]
                        if with_bias:
                            nc.vector.tensor_scalar_add(
                                out=sl, in0=state["acc"],
                                scalar1=b_sb[:, ct:ct + 1])
                        else:
                            nc.vector.tensor_copy(out=sl, in_=state["acc"])
                return emit
            for hc in range(HC):
                fill_q.append(mk(hc))

        def emit_v_now(tt):
            acc = fill_pool.tile([P, G], F32, tag="acc", name="vacc")
            for hc in range(HC):
                nc.tensor.matmul(
                    acc,
                    lhsT=xT_sb[:, hc, tt * P:(tt + 1) * P],
                    rhs=wv_sb[:, hc, :],
                    start=(hc == 0),
                    stop=(not with_bias and hc == HC - 1))
            if with_bias:
                nc.tensor.matmul(
                    acc, lhsT=ones_col, rhs=bv_row, start=False, stop=True)
            nc.vector.tensor_copy(
                out=v_aug[:, tt, :, 0:HD],
                in_=acc.rearrange("p (h d) -> p h d", h=NHL))

        def enq_o_group(tt, no):
            state = {}

            def mk(cc):
                def emit():
                    if cc == 0:
                        state["acc"] = fill_pool.tile(
                            [P, W], F32, tag="acc", name="oacc")
                    nc.tensor.matmul(
                        state["acc"],
                        lhsT=ctxT[:, cc, tt * P:(tt + 1) * P],
                        rhs=wo_sb[:, cc, no * W:(no + 1) * W],
                        start=(cc == 0), stop=(cc == CT - 1))
                    if cc == CT - 1:
                        ot = osb.tile([P, W], BF16)
                        nc.vector.tensor_copy(out=ot, in_=state["acc"])
                        nc.sync.dma_start(
                            out=out[tt * P:(tt + 1) * P,
                                    no * W:(no + 1) * W],
                            in_=ot)
                return emit
            for cc in range(CT):
                fill_q.append(mk(cc))

        def drain_fill(n):
            for _ in range(n):
                if not fill_q:
                    return
                fill_q.popleft()()

        # ---- lead-in: kT + first q window of pair 0 ----
        for nq in range(NQW):
            enq_qk_group(1, 0, nq)
        enq_qk_group(0, 0, 0)
        drain_fill(len(fill_q))

        exp_pool = ctx.enter_context(tc.tile_pool(name="expp", bufs=4))
        norm_pool = ctx.enter_context(tc.tile_pool(name="normp", bufs=2))

        # ---- pipelined normalization stages ----
        # stage1 (at window end): fast psum-releasing copies (den rows to
        # sbuf, raw ctx to ctxT).  stage2 (start of next window): reciprocal
        # + bf16 cast + DRAM round-trip broadcast.  stage3 (one window
        # later): the in-place normalize multiply.  Keeps the slow
        # reciprocal and DMA latencies off the ctx psum-slot critical path.
        def norm_stage2(e):
            rcp2f = norm_pool.tile([33, W], F32, tag="rcpf")
            nc.vector.reciprocal(out=rcp2f, in_=e["den"])
            rcp2 = norm_pool.tile([33, W], BF16, tag="rcpb")
            nc.vector.tensor_copy(out=rcp2, in_=rcp2f)
            w_idx = e["w_idx"]
            nc.sync.dma_start(
                out=rcp_dram[w_idx:w_idx + 1, :], in_=rcp2[0:1, :])
            nc.sync.dma_start(
                out=rcp_dram[w_idx + 1:w_idx + 2, :], in_=rcp2[32:33, :])
            bcast = norm_pool.tile([P, W], BF16, tag="bcast")
            for hh in range(2):
                row = rcp_dram[w_idx + hh:w_idx + hh + 1, :]
                nc.sync.dma_start(
                    out=bcast[hh * HD:(hh + 1) * HD, :],
                    in_=bass.AP(tensor=row.tensor, offset=row.offset,
                                ap=[[0, HD], [1, W]]))
            e["bcast"] = bcast

        def norm_stage3(e):
            sl = ctxT[:, e["pair"], e["q0"]:e["q0"] + W]
            nc.vector.tensor_mul(out=sl, in0=sl, in1=e["bcast"])

        norm_chain = deque()

        def pump_norm():
            if len(norm_chain) >= 2 and "bcast" not in norm_chain[-2]:
                norm_stage2(norm_chain[-2])
            if len(norm_chain) >= 3:
                norm_stage3(norm_chain.popleft())

        # ---- main attention loop: pair -> q-window -> kc ----
        for pair in range(CT):
            h0 = 2 * pair
            for qw in range(NQW):
                q0 = qw * W
                w_idx = (pair * NQW + qw) * 2
                pump_norm()
                ctx0 = ctx_pool.tile([HD + 1, W], F32, tag="c0", name="ctx0")
                ctx1 = ctx_pool.tile([HD + 1, W], F32, tag="c1", name="ctx1")

                # enqueue this window's drip work (deadline: all of it
                # drains before the window ends).  Own next-qT; the next
                # pair's kT spread over the middle windows; its first qT in
                # the last window.
                if qw + 1 < NQW:
                    enq_qk_group(0, pair, qw + 1)
                if pair + 1 < CT:
                    kt_sched = {1: (0, 1), 2: (2, 3)} if pair == 0 else \
                        {0: (0,), 1: (1,), 2: (2, 3)}
                    for nq in kt_sched.get(qw, ()):
                        enq_qk_group(1, pair + 1, nq)
                    if qw == NQW - 1:
                        enq_qk_group(0, pair + 1, 0)

                budget = max(1, -(-len(fill_q) // KC))

                first_win = (pair == 0 and qw == 0)
                prev_ex = None
                for kc in range(KC):
                    if first_win:
                        # v[kc] must exist before AV(kc) next iteration.
                        # fill groups must not interleave with v groups on
                        # the shared psum "acc" slots, so the one queued qT
                        # group drains as a single lump between two of them.
                        emit_v_now(kc)
                        if kc == 5:
                            drain_fill(HC)
                    sc = sc_pool.tile([P, 2, W], F32, tag="sc", name="sc")
                    nc.tensor.matmul(
                        sc[:, 0, :],
                        lhsT=kT[0:HD, pair, kc * P:(kc + 1) * P],
                        rhs=qT[0:HD, pair, q0:q0 + W],
                        start=True, stop=True)
                    nc.tensor.matmul(
                        sc[:, 1, :],
                        lhsT=kT[HD:P, pair, kc * P:(kc + 1) * P],
                        rhs=qT[HD:P, pair, q0:q0 + W],
                        start=True, stop=True)
                    if not first_win:
                        drain_fill(budget)
                    if prev_ex is not None:
                        pex, pkc = prev_ex
                        nc.tensor.matmul(
                            ctx0, lhsT=v_aug[:, pkc, h0, :],
                            rhs=pex[:, 0, :],
                            start=(pkc == 0), stop=False)
                        nc.tensor.matmul(
                            ctx1, lhsT=v_aug[:, pkc, h0 + 1, :],
                            rhs=pex[:, 1, :],
                            start=(pkc == 0), stop=False)
                    ex = exp_pool.tile([P, 2, W], BF16, tag="ex", name="ex")
                    nc.scalar.activation(
                        out=ex, in_=sc,
                        func=mybir.ActivationFunctionType.Exp,
                        scale=SCALE)
                    prev_ex = (ex, kc)

                pex, pkc = prev_ex
                nc.tensor.matmul(
                    ctx0, lhsT=v_aug[:, pkc, h0, :], rhs=pex[:, 0, :],
                    start=False, stop=True)
                nc.tensor.matmul(
                    ctx1, lhsT=v_aug[:, pkc, h0 + 1, :], rhs=pex[:, 1, :],
                    start=False, stop=True)

                # ---- window wrap-up (stage1): fast psum-slot-releasing
                # copies only; the slow reciprocal + broadcast chain is
                # deferred into the next windows via pump_norm ----
                den = norm_pool.tile([33, W], F32, tag="den")
                nc.vector.tensor_copy(out=den[0:1, :], in_=ctx0[HD:HD + 1, :])
                nc.vector.tensor_copy(
                    out=den[32:33, :], in_=ctx1[HD:HD + 1, :])
                nc.vector.tensor_copy(
                    out=ctxT[0:HD, pair, q0:q0 + W], in_=ctx0[0:HD, :])
                nc.vector.tensor_copy(
                    out=ctxT[HD:P, pair, q0:q0 + W], in_=ctx1[0:HD, :])
                norm_chain.append(
                    {"den": den, "w_idx": w_idx, "pair": pair, "q0": q0})

        # ---- flush the norm pipeline ----
        for e in norm_chain:
            if "bcast" not in e:
                norm_stage2(e)
        while norm_chain:
            norm_stage3(norm_chain.popleft())

        # ---- tail: o-projection (needs all pairs normalized) ----
        for qw in range(NQW):
            for tt in range(4 * qw, 4 * qw + 4):
                for no in range(H // W):
                    enq_o_group(tt, no)
        drain_fill(len(fill_q))


def build_program(with_bias=True):
    if with_bias in _NC_CACHE:
        return _NC_CACHE[with_bias]
    nc = bacc.Bacc("TRN2", debug=False, num_devices=8)
    xT = nc.dram_tensor("xT", [H, S], BF16, kind="ExternalInput").ap()
    wq = nc.dram_tensor("wq", [H, G], BF16, kind="ExternalInput").ap()
    wk = nc.dram_tensor("wk", [H, G], BF16, kind="ExternalInput").ap()
    wv = nc.dram_tensor("wv", [H, G], BF16, kind="ExternalInput").ap()
    wo = nc.dram_tensor("wo", [G, H], BF16, kind="ExternalInput").ap()
    bq = nc.dram_tensor("bq", [G], F32, kind="ExternalInput").ap()
    bk = nc.dram_tensor("bk", [G], F32, kind="ExternalInput").ap()
    bv = nc.dram_tensor("bv", [G], F32, kind="ExternalInput").ap()
    out = nc.dram_tensor("out", [S, H], BF16, kind="ExternalOutput").ap()
    rcp_dram = nc.dram_tensor("rcp_scratch", [NWH, W], BF16).ap()
    with tile.TileContext(nc) as tc:
        _emit(nc, tc, (xT, wq, wk, wv, wo, bq, bk, bv, out, rcp_dram),
              with_bias)
    nc.compile()
    _NC_CACHE[with_bias] = nc
    return nc


def make_in_maps(x, wq, bq, wk, bk, wv, bv, wo, bo):
    import ml_dtypes
    bf = ml_dtypes.bfloat16
    x = np.asarray(x, dtype=np.float32)
    wq, wk, wv, wo = (np.asarray(w, np.float32).astype(bf)
                      for w in (wq, wk, wv, wo))
    in_maps = []
    for c in range(8):
        b, g = divmod(c, 2)
        sl = slice(g * G, (g + 1) * G)
        in_maps.append({
            "xT": np.ascontiguousarray(x[b].T.astype(bf)),
            "wq": np.ascontiguousarray(wq[:, sl]),
            "wk": np.ascontiguousarray(wk[:, sl]),
            "wv": np.ascontiguousarray(wv[:, sl]),
            "wo": np.ascontiguousarray(wo[sl, :]),
            "bq": np.ascontiguousarray(np.asarray(bq, np.float32)[sl]),
            "bk": np.ascontiguousarray(np.asarray(bk, np.float32)[sl]),
            "bv": np.ascontiguousarray(np.asarray(bv, np.float32)[sl]),
        })
    return in_maps


def gather_out(results, bo):
    bo = np.asarray(bo, dtype=np.float32)
    out = np.empty((B, S, H), dtype=np.float32)
    for b in range(B):
        out[b] = (results[2 * b]["out"].astype(np.float32)
                  + results[2 * b + 1]["out"].astype(np.float32) + bo)
    return out


def kernel(x, wq, bq, wk, bk, wv, bv, wo, bo, trace=False):
    with_bias = any(
        np.any(np.asarray(b)) for b in (bq, bk, bv))
    nc = build_program(with_bias)
    in_maps = make_in_maps(x, wq, bq, wk, bk, wv, bv, wo, bo)
    r = run_bass_kernel_spmd(nc, in_maps, list(range(8)), trace=trace)
    out = gather_out(r.results, bo)
    if trace:
        kernel.last_exec_time_ns = r.exec_time_ns
        kernel.last_results = r
    return out
